# revision 6
# baseline (speedup 1.0000x reference)
# MoE (top-2 routed experts + shared expert SwiGLU) on 8 TRN2 NeuronCores.
#
# Sharding: expert-parallel. Core e owns expert e's FFN weights and processes
# the tokens routed to expert e (capacity factor 1.0 = 512 slots; the few
# overflow tokens are computed host-side with bit-matching bf16 math); the
# shared expert runs data-parallel (each core takes T/8 tokens with replicated
# shared weights). Routing (sigmoid gate -> top-2 -> stable sort by expert)
# is part of the host-side sharding step: it decides which token goes to
# which core, exactly mirroring the reference's jax ops so expert selection
# is bit-identical. All FFN GEMMs (99.9% of FLOPs) run on device in bf16
# with fp32 PSUM accumulation, matching the reference's bf16 expert compute.
#
# Device layout: tokens live on the matmul free dim (everything pre-transposed
# host-side), weights stream as [128, free] k-tiles used as lhsT slices.
# GEMM1 weights are packed in 4 hidden-quarters of 4 PSUM banks each so
# consecutive quarters double-buffer through PSUM (no eviction bubbles) and
# weight DMA deadlines are pipelined.
import os
import sys
import tempfile

import numpy as np
import ml_dtypes

for _p in ("/opt/trn_rl_repo", "/root/.axon_site/_ro/trn_rl_repo"):
    if os.path.isdir(_p) and _p not in sys.path:
        sys.path.append(_p)

BF16 = ml_dtypes.bfloat16

P = 128
D = 2048          # model dim
H = 1024          # ffn hidden dim
T = 2048          # batch*seq tokens
E = 8             # experts == cores
TOPK = 2
C = 512           # per-expert token capacity (factor 1.0; host computes overflow)
S = T // 8        # shared-expert tokens per core
KH = H // P       # 8 k-tiles over H (GEMM2 contraction)
F = 4             # D-fold factor: d = f*(D//F) + r; fattens DMA lines 4x
DR = D // F       # 512 folded rows
KF = DR // P      # 4 row-tiles over folded D
G = 4             # GEMM1 hidden-quarters (each = 2 w1-tiles + 2 w3-tiles)
JG = 2            # 128-row hidden tiles per matrix per quarter
HQ = H // G       # 256 hidden rows per quarter

_COMPILED = {}     # build_key -> (nc, tmpdir)
LAST_RESULTS = None  # BassKernelResults of the most recent device run (for test.py)


def _ensure_axon_hooks():
    """This image's antenv lacks axon_hooks, which run_bass_kernel_spmd
    imports unconditionally when tracing. Provide it, wired to the
    libaxon_pjrt ctypes NTFF hook when available."""
    try:
        import antenv.axon_hooks  # noqa: F401
        return
    except ImportError:
        pass
    import types

    try:
        import antenv
    except ImportError:
        return
    mod = types.ModuleType("antenv.axon_hooks")
    holder = {"hook": None}
    mod.set_axon_ntff_profile_hook = lambda h: holder.__setitem__("hook", h)
    mod.get_axon_ntff_profile_hook = lambda: holder["hook"]
    sys.modules["antenv.axon_hooks"] = mod
    antenv.axon_hooks = mod
    try:
        from trn_agent_boot.trn_boot import _ntff_profile_via_ctypes

        hook = _ntff_profile_via_ctypes("/opt/axon/libaxon_pjrt.so")
        if hook is not None:
            mod.set_axon_ntff_profile_hook(hook)
    except Exception:
        pass


_ensure_axon_hooks()


def _build_nc():
    import concourse.bass as bass  # noqa: F401
    import concourse.tile as tile
    from concourse import bacc, mybir

    bf = mybir.dt.bfloat16
    f32 = mybir.dt.float32
    act = mybir.ActivationFunctionType

    nc = bacc.Bacc("TRN2", target_bir_lowering=False, debug=False, num_devices=8)

    # Folded-D DRAM layouts (see kernel() host packing):
    #   x:   [DR, F*n] — F fold-blocks of n token columns
    #   w13: per quarter g in 0..3: [DR, F*512]; within fold block f,
    #        cols [0:256) are w1's hidden-quarter, [256:512) are w3's.
    # Folding multiplies DMA line length by F (4), cutting per-packet DMA
    # overhead; the contraction over D becomes a loop over (row-tile, fold).
    xr = nc.dram_tensor("xr", [DR, F * C], bf, kind="ExternalInput").ap()
    xs = nc.dram_tensor("xs", [DR, F * S], bf, kind="ExternalInput").ap()
    w13 = nc.dram_tensor("w13", [G, DR, F * 2 * HQ], bf, kind="ExternalInput").ap()
    w2 = nc.dram_tensor("w2", [H, D], bf, kind="ExternalInput").ap()
    sw13 = nc.dram_tensor("sw13", [G, DR, F * 2 * HQ], bf, kind="ExternalInput").ap()
    sw2 = nc.dram_tensor("sw2", [H, D], bf, kind="ExternalInput").ap()
    # Outputs use the same folded layout as x (unfolded host-side). Both are
    # bf16: routed math is bf16 anyway; the shared output rounds f32 PSUM to
    # bf16 (well within tolerance) and halves the tail-store traffic.
    o_r = nc.dram_tensor("o_r", [DR, F * C], bf, kind="ExternalOutput").ap()
    o_s = nc.dram_tensor("o_s", [DR, F * S], bf, kind="ExternalOutput").ap()

    with tile.TileContext(nc) as tc:
        with (
            tc.tile_pool(name="xp", bufs=9) as xpool,
            tc.tile_pool(name="wg", bufs=12) as wgpool,
            tc.tile_pool(name="w2p", bufs=10) as w2pool,
            tc.tile_pool(name="hp", bufs=18) as hpool,
            tc.tile_pool(name="op", bufs=3) as opool,
            tc.tile_pool(name="ps", bufs=8, space="PSUM") as pspool,
        ):
            def warmup():
                # Dummy matmuls bridge the gap between the PE's program start
                # (~7.7us, after the framework preamble) and the first input
                # blocks completing (~11-12.3us: early transfers pay a ~3-4us
                # completion latency while the DMA engines still drain the
                # program-load static DMAs), and cover the ~3.4us HAM
                # clock-gate ramp so real matmuls start at 2.4 GHz.
                zt = hpool.tile([P, 256], bf, tag="h", name="warm_x")
                nc.vector.memset(zt[:], 0.0)
                pw = pspool.tile([P, 256], f32, tag="ps", name="warm_ps")
                for it in range(8):
                    nc.tensor.matmul(
                        pw[:], zt[:, :P], zt[:], start=(it == 0), stop=(it == 7)
                    )

            def ffn(x_dram, n, w13_dram, w2_dram, out_dram, out_dt,
                    first_phase=False, split_out=False):
                x_sb = [None] * KF
                wq_sb = [[None] * KF for _ in range(G)]
                w2_sb = [None] * KH
                FW = F * 2 * HQ  # 2048 weight cols per quarter k-tile

                if first_phase:
                    # Fine-grained opening split across BOTH HWDGE rings
                    # (sync + scalar) in PE consumption order. The very first
                    # k-tile goes per-fold-block (128KB pieces): during the
                    # DMA-engine ramp the first transfer's completion latency
                    # is several us, so the first pieces must be small for
                    # real matmuls to start right as the warmup ends (~9.6us).
                    xh = F * n // 2
                    wh = FW // 2
                    for kt in range(KF):
                        xt0 = xpool.tile([P, F * n], bf, tag="x",
                                         name=f"x_{kt}")
                        wt0 = wgpool.tile([P, FW], bf, tag="wg",
                                          name=f"wg0_{kt}")
                        rows = slice(kt * P, (kt + 1) * P)
                        if kt == 0:
                            for f in range(F):
                                ring = nc.sync if f < 2 else nc.scalar
                                ring.dma_start(
                                    xt0[:, f * n:(f + 1) * n],
                                    x_dram[rows, f * n:(f + 1) * n])
                                ring.dma_start(
                                    wt0[:, f * 2 * HQ:(f + 1) * 2 * HQ],
                                    w13_dram[0, rows,
                                             f * 2 * HQ:(f + 1) * 2 * HQ])
                        else:
                            nc.sync.dma_start(xt0[:, :xh], x_dram[rows, :xh])
                            nc.sync.dma_start(wt0[:, :wh],
                                              w13_dram[0, rows, :wh])
                            nc.scalar.dma_start(xt0[:, xh:], x_dram[rows, xh:])
                            nc.scalar.dma_start(wt0[:, wh:],
                                                w13_dram[0, rows, wh:])
                        x_sb[kt] = xt0
                        wq_sb[0][kt] = wt0
                else:
                    # shared-phase x rides the scalar ring (idle mid-kernel);
                    # weights continue on sync in deadline order
                    for kt in range(KF):
                        t = xpool.tile([P, F * n], bf, tag="x", name=f"xs_{kt}")
                        nc.scalar.dma_start(t[:], x_dram[kt * P:(kt + 1) * P, :])
                        x_sb[kt] = t
                    for kt in range(KF):
                        w = wgpool.tile([P, FW], bf, tag="wg", name=f"sq0_{kt}")
                        nc.sync.dma_start(w[:], w13_dram[0, kt * P:(kt + 1) * P, :])
                        wq_sb[0][kt] = w
                for g in range(1, G):
                    for kt in range(KF):
                        w = wgpool.tile([P, FW], bf, tag="wg",
                                        name=f"wg{g}_{kt}")
                        nc.sync.dma_start(w[:], w13_dram[g, kt * P:(kt + 1) * P, :])
                        wq_sb[g][kt] = w
                for k2 in range(KH):
                    t = w2pool.tile([P, D], bf, tag="w2", name=f"w2_{k2}")
                    nc.sync.dma_start(t[:], w2_dram[k2 * P:(k2 + 1) * P, :])
                    w2_sb[k2] = t

                # GEMM1 over 4 quarters x 4 PSUM banks: quarter g+1's matmuls
                # run in banks 4..7 while quarter g's banks 0..3 evict through
                # ACT silu + DVE mul — no PSUM switch bubbles.
                h_sb = [None] * KH
                for g in range(G):
                    pg1 = [
                        pspool.tile([P, n], f32, tag="ps", name=f"pg1_{g}_{j}")
                        for j in range(JG)
                    ]
                    pg3 = [
                        pspool.tile([P, n], f32, tag="ps", name=f"pg3_{g}_{j}")
                        for j in range(JG)
                    ]
                    # f-inner ordering: 4 consecutive accumulations into the
                    # same PSUM bank before switching banks. Bank-cycling
                    # every MM costs ~4ns/MM in PE micro-idles (same-bank
                    # runs measure at clean N/2.4GHz+2.5ns pitch, as GEMM2's
                    # 8-long same-bank runs show). Per-bank accumulation
                    # order over (kt, f) is unchanged.
                    for kt in range(KF):
                        wt = wq_sb[g][kt]
                        xt_ = x_sb[kt]
                        for j in range(JG):
                            for w_off, pg in ((0, pg1), (HQ, pg3)):
                                for f in range(F):
                                    nc.tensor.matmul(
                                        pg[j][:],
                                        wt[:, f * 2 * HQ + w_off + j * P:
                                           f * 2 * HQ + w_off + (j + 1) * P],
                                        xt_[:, f * n:(f + 1) * n],
                                        start=(kt == 0 and f == 0),
                                        stop=(kt == KF - 1 and f == F - 1),
                                    )
                    for j in range(JG):
                        s_sb = hpool.tile([P, n], bf, tag="h")
                        nc.scalar.activation(s_sb[:], pg1[j][:], act.Silu)
                        h = hpool.tile([P, n], bf, tag="h")
                        nc.vector.tensor_mul(h[:], s_sb[:], pg3[j][:])
                        h_sb[g * JG + j] = h

                for gr in range(KF):
                    o = opool.tile([P, F * n], out_dt, tag="o",
                                   name=f"o_{gr}")
                    for fd in range(F):
                        om = fd * KF + gr  # d rows [om*P, om*P+P)
                        po = pspool.tile([P, n], f32, tag="ps",
                                         name=f"po_{om}")
                        for kt in range(KH):
                            nc.tensor.matmul(
                                po[:],
                                w2_sb[kt][:, om * P:(om + 1) * P],
                                h_sb[kt][:],
                                start=(kt == 0), stop=(kt == KH - 1),
                            )
                        nc.vector.tensor_copy(
                            o[:, fd * n:(fd + 1) * n], po[:]
                        )
                        if split_out:
                            # last phase: stream each fold block out as soon
                            # as it is evicted — tail latency beats line
                            # efficiency at kernel end
                            nc.sync.dma_start(
                                out_dram[gr * P:(gr + 1) * P,
                                         fd * n:(fd + 1) * n],
                                o[:, fd * n:(fd + 1) * n],
                            )
                    if not split_out:
                        nc.sync.dma_start(
                            out_dram[gr * P:(gr + 1) * P, :], o[:]
                        )

            warmup()
            ffn(xr, C, w13, w2, o_r, bf, first_phase=True)
            ffn(xs, S, sw13, sw2, o_s, bf, split_out=True)

    nc.compile()
    return nc


def _get_compiled():
    if "nc" not in _COMPILED:
        _COMPILED["nc"] = _build_nc()
        _COMPILED["tmpdir"] = tempfile.mkdtemp(prefix="moe_bass_")
    return _COMPILED["nc"], _COMPILED["tmpdir"]


def _route_host(x, gate, expert_bias):
    """Reference-exact routing on CPU jax: scores, top-2 selection, stable
    sort by expert. Returns (token_idx, expert_ids, scores_sorted) in
    sorted-slot order."""
    import jax
    import jax.numpy as jnp

    cpu = jax.devices("cpu")[0]
    with jax.default_device(cpu):
        xt = jnp.asarray(x.reshape(-1, D))
        scores = jax.nn.sigmoid((xt @ jnp.asarray(gate).T).astype(jnp.float32))
        _, sel = jax.lax.top_k(scores + jnp.asarray(expert_bias)[None, :], TOPK)
        top_scores = jnp.take_along_axis(scores, sel, axis=1) * 1.0
        flat_sel = sel.reshape(-1)
        order = jnp.argsort(flat_sel, stable=True)
        scores_sorted = top_scores.reshape(-1)[order]
        expert_ids = flat_sel[order]
    order = np.asarray(order)
    return (
        order // TOPK,
        np.asarray(expert_ids),
        np.asarray(scores_sorted, dtype=np.float32),
        order,
    )


def _silu32(v):
    return v / (1.0 + np.exp(-v))


def fold_x(x_t):
    # x_t: [D, n] f32/bf16 -> [DR, F*n] bf16, fold-major column blocks
    n = x_t.shape[1]
    xf = np.asarray(x_t).reshape(F, DR, n)
    return np.ascontiguousarray(
        xf.transpose(1, 0, 2).reshape(DR, F * n).astype(BF16)
    )


def unfold_x(arr_f, n_cols):
    # inverse of fold_x: [DR, F*n_cols] -> [D, n_cols]
    out = np.empty((D, n_cols), dtype=arr_f.dtype)
    for f in range(F):
        out[f * DR:(f + 1) * DR] = arr_f[:, f * n_cols:(f + 1) * n_cols]
    return out


def fold_w13(a1, a3):
    # -> [G, DR, F*2*HQ]: per hidden-quarter g, fold-major column blocks,
    # each block = [w1 quarter | w3 quarter]
    out = np.empty((G, DR, F * 2 * HQ), dtype=BF16)
    for g in range(G):
        wg = np.concatenate(
            [a1.T[:, g * HQ:(g + 1) * HQ], a3.T[:, g * HQ:(g + 1) * HQ]],
            axis=1,
        )  # [D, 2*HQ]
        out[g] = wg.reshape(F, DR, 2 * HQ).transpose(1, 0, 2).reshape(
            DR, F * 2 * HQ
        )
    return out


def _overflow_slots_numpy(xb_rows, w1e, w2e, w3e):
    """Exact-math fallback for expert token counts beyond capacity C:
    reproduce the reference's bf16 FFN math in numpy for those rows."""
    a = xb_rows.astype(np.float32)
    g1 = (a @ w1e.astype(BF16).astype(np.float32).T).astype(BF16)
    g3 = (a @ w3e.astype(BF16).astype(np.float32).T).astype(BF16)
    h = (_silu32(g1.astype(np.float32))).astype(BF16).astype(np.float32)
    h = (h * g3.astype(np.float32)).astype(BF16)
    return (h.astype(np.float32) @ w2e.astype(BF16).astype(np.float32).T).astype(
        BF16
    ).astype(np.float32)


def kernel(x, gate, expert_bias, w1, w2, w3, shared_w1, shared_w2, shared_w3):
    global LAST_RESULTS
    from concourse.bass_utils import run_bass_kernel_spmd

    x = np.asarray(x, dtype=np.float32)
    gate = np.asarray(gate, dtype=np.float32)
    expert_bias = np.asarray(expert_bias, dtype=np.float32)
    w1 = np.asarray(w1, dtype=np.float32)
    w2 = np.asarray(w2, dtype=np.float32)
    w3 = np.asarray(w3, dtype=np.float32)
    shared_w1 = np.asarray(shared_w1, dtype=np.float32)
    shared_w2 = np.asarray(shared_w2, dtype=np.float32)
    shared_w3 = np.asarray(shared_w3, dtype=np.float32)

    token_idx, expert_ids, scores_sorted, order = _route_host(x, gate, expert_bias)
    xt = x.reshape(T, D)

    counts = np.bincount(expert_ids, minlength=E)
    offs = np.concatenate([[0], np.cumsum(counts)])

    # Routed tokens, scaled by their gate score then rounded to bf16 exactly
    # like the reference's `routed.astype(bfloat16)`.
    routed_b = (xt[token_idx] * scores_sorted[:, None]).astype(BF16)

    # Shared weights are identical on every core.
    sw13_t = fold_w13(shared_w1, shared_w3)
    sw2_t = np.ascontiguousarray(shared_w2.T.astype(BF16))
    xt_b = xt.astype(BF16)

    in_maps = []
    for e in range(E):
        lo, hi = offs[e], offs[e + 1]
        n_e = min(hi - lo, C)
        xr_t = np.zeros((D, C), dtype=BF16)
        xr_t[:, :n_e] = routed_b[lo:lo + n_e].T
        in_maps.append(
            {
                "xr": fold_x(xr_t),
                "xs": fold_x(xt_b[e * S:(e + 1) * S].T),
                "w13": fold_w13(w1[e], w3[e]),
                "w2": np.ascontiguousarray(w2[e].T.astype(BF16)),
                "sw13": sw13_t,
                "sw2": sw2_t,
            }
        )

    nc, _ = _get_compiled()
    # fresh tmpdir per call: NTFF profile artifacts collide on reuse
    tmpdir = tempfile.mkdtemp(prefix="moe_bass_")
    res = run_bass_kernel_spmd(nc, in_maps, core_ids=list(range(E)), tmpdir=tmpdir)
    LAST_RESULTS = res

    # Reassemble: shared output slices + scatter-add of routed outputs.
    out = np.empty((T, D), dtype=np.float32)
    for e in range(E):
        out[e * S:(e + 1) * S] = unfold_x(res.results[e]["o_s"], S).T

    out_r = np.empty((T * TOPK, D), dtype=np.float32)
    for e in range(E):
        lo, hi = offs[e], offs[e + 1]
        n_e = min(hi - lo, C)
        o_r_e = unfold_x(res.results[e]["o_r"], C)
        out_r[lo:lo + n_e] = o_r_e[:, :n_e].T.astype(np.float32)
        if hi - lo > C:  # capacity overflow: exact numpy fallback
            rows = routed_b[lo + C:hi]
            out_r[lo + C:hi] = _overflow_slots_numpy(rows, w1[e], w2[e], w3[e])

    # slot s (sorted order) came from original flat slot order[s]; invert so
    # each token's two expert outputs can be summed with one gather.
    pos = np.empty(T * TOPK, dtype=np.int64)
    pos[order] = np.arange(T * TOPK)
    out += out_r[pos].reshape(T, TOPK, D).sum(axis=1)

    return out.reshape(4, 512, D)


# revision 8
# speedup vs baseline: 1.1892x; 1.1892x over previous
# MoE (top-2 routed experts + shared expert SwiGLU) on 8 TRN2 NeuronCores.
#
# Sharding: expert-parallel. Core e owns expert e's FFN weights and processes
# the tokens routed to expert e (capacity factor 1.0 = 512 slots; the few
# overflow tokens are computed host-side with bit-matching bf16 math); the
# shared expert runs data-parallel (each core takes T/8 tokens with replicated
# shared weights). Routing (sigmoid gate -> top-2 -> stable sort by expert)
# is part of the host-side sharding step: it decides which token goes to
# which core, exactly mirroring the reference's jax ops so expert selection
# is bit-identical. All FFN GEMMs (99.9% of FLOPs) run on device in bf16
# with fp32 PSUM accumulation, matching the reference's bf16 expert compute.
#
# Device layout: tokens live on the matmul free dim (everything pre-transposed
# host-side), weights stream as [128, free] k-tiles used as lhsT slices.
# GEMM1 weights are packed in 4 hidden-quarters of 4 PSUM banks each so
# consecutive quarters double-buffer through PSUM (no eviction bubbles) and
# weight DMA deadlines are pipelined.
import os
import sys
import tempfile

import numpy as np
import ml_dtypes

for _p in ("/opt/trn_rl_repo", "/root/.axon_site/_ro/trn_rl_repo"):
    if os.path.isdir(_p) and _p not in sys.path:
        sys.path.append(_p)

BF16 = ml_dtypes.bfloat16

P = 128
D = 2048          # model dim
H = 1024          # ffn hidden dim
T = 2048          # batch*seq tokens
E = 8             # experts == cores
TOPK = 2
C = 512           # per-expert token capacity (factor 1.0; host computes overflow)
S = T // 8        # shared-expert tokens per core
KH = H // P       # 8 k-tiles over H (GEMM2 contraction)
F = 4             # D-fold factor: d = f*(D//F) + r; fattens DMA lines 4x
DR = D // F       # 512 folded rows
KF = DR // P      # 4 row-tiles over folded D
G = 4             # GEMM1 hidden-quarters (each = 2 w1-tiles + 2 w3-tiles)
JG = 2            # 128-row hidden tiles per matrix per quarter
HQ = H // G       # 256 hidden rows per quarter

_COMPILED = {}     # build_key -> (nc, tmpdir)
LAST_RESULTS = None  # BassKernelResults of the most recent device run (for test.py)


def _ensure_axon_hooks():
    """This image's antenv lacks axon_hooks, which run_bass_kernel_spmd
    imports unconditionally when tracing. Provide it, wired to the
    libaxon_pjrt ctypes NTFF hook when available."""
    try:
        import antenv.axon_hooks  # noqa: F401
        return
    except ImportError:
        pass
    import types

    try:
        import antenv
    except ImportError:
        return
    mod = types.ModuleType("antenv.axon_hooks")
    holder = {"hook": None}
    mod.set_axon_ntff_profile_hook = lambda h: holder.__setitem__("hook", h)
    mod.get_axon_ntff_profile_hook = lambda: holder["hook"]
    sys.modules["antenv.axon_hooks"] = mod
    antenv.axon_hooks = mod
    try:
        from trn_agent_boot.trn_boot import _ntff_profile_via_ctypes

        hook = _ntff_profile_via_ctypes("/opt/axon/libaxon_pjrt.so")
        if hook is not None:
            mod.set_axon_ntff_profile_hook(hook)
    except Exception:
        pass


_ensure_axon_hooks()


def _build_nc():
    import concourse.bass as bass  # noqa: F401
    import concourse.tile as tile
    from concourse import bacc, mybir

    bf = mybir.dt.bfloat16
    f32 = mybir.dt.float32
    act = mybir.ActivationFunctionType

    nc = bacc.Bacc("TRN2", target_bir_lowering=False, debug=False, num_devices=8)

    # Folded-D DRAM layouts (see kernel() host packing):
    #   x:   [DR, F*n] — F fold-blocks of n token columns
    #   w13: per quarter g in 0..3: [DR, F*512]; within fold block f,
    #        cols [0:256) are w1's hidden-quarter, [256:512) are w3's.
    # Folding multiplies DMA line length by F (4), cutting per-packet DMA
    # overhead; the contraction over D becomes a loop over (row-tile, fold).
    xr = nc.dram_tensor("xr", [DR, F * C], bf, kind="ExternalInput").ap()
    xs = nc.dram_tensor("xs", [DR, F * S], bf, kind="ExternalInput").ap()
    w13 = nc.dram_tensor("w13", [G, DR, F * 2 * HQ], bf, kind="ExternalInput").ap()
    w2 = nc.dram_tensor("w2", [H, D], bf, kind="ExternalInput").ap()
    sw13 = nc.dram_tensor("sw13", [G, DR, F * 2 * HQ], bf, kind="ExternalInput").ap()
    sw2 = nc.dram_tensor("sw2", [H, D], bf, kind="ExternalInput").ap()
    # Outputs use the same folded layout as x (unfolded host-side). Both are
    # bf16: routed math is bf16 anyway; the shared output rounds f32 PSUM to
    # bf16 (well within tolerance) and halves the tail-store traffic.
    o_r = nc.dram_tensor("o_r", [DR, F * C], bf, kind="ExternalOutput").ap()
    o_s = nc.dram_tensor("o_s", [DR, F * S], bf, kind="ExternalOutput").ap()

    with tile.TileContext(nc) as tc:
        with (
            tc.tile_pool(name="xp", bufs=9) as xpool,
            tc.tile_pool(name="wg", bufs=12) as wgpool,
            tc.tile_pool(name="w2p", bufs=10) as w2pool,
            tc.tile_pool(name="hp", bufs=18) as hpool,
            tc.tile_pool(name="op", bufs=3) as opool,
            tc.tile_pool(name="ps", bufs=8, space="PSUM") as pspool,
        ):
            def warmup():
                # Dummy matmuls bridge the gap between the PE's program start
                # (~7.7us, after the framework preamble) and the first input
                # blocks completing (~11-12.3us: early transfers pay a ~3-4us
                # completion latency while the DMA engines still drain the
                # program-load static DMAs), and cover the ~3.4us HAM
                # clock-gate ramp so real matmuls start at 2.4 GHz.
                zt = hpool.tile([P, 256], bf, tag="h", name="warm_x")
                nc.vector.memset(zt[:], 0.0)
                pw = pspool.tile([P, 256], f32, tag="ps", name="warm_ps")
                for it in range(8):
                    nc.tensor.matmul(
                        pw[:], zt[:, :P], zt[:], start=(it == 0), stop=(it == 7)
                    )

            def ffn(x_dram, n, w13_dram, w2_dram, out_dram, out_dt,
                    first_phase=False, split_out=False):
                x_sb = [None] * KF
                wq_sb = [[None] * KF for _ in range(G)]
                w2_sb = [None] * KH
                FW = F * 2 * HQ  # 2048 weight cols per quarter k-tile

                if first_phase:
                    # Fine-grained opening split across BOTH HWDGE rings
                    # (sync + scalar) in PE consumption order. The very first
                    # k-tile goes per-fold-block (128KB pieces): during the
                    # DMA-engine ramp the first transfer's completion latency
                    # is several us, so the first pieces must be small for
                    # real matmuls to start right as the warmup ends (~9.6us).
                    xh = F * n // 2
                    wh = FW // 2
                    for kt in range(KF):
                        xt0 = xpool.tile([P, F * n], bf, tag="x",
                                         name=f"x_{kt}")
                        wt0 = wgpool.tile([P, FW], bf, tag="wg",
                                          name=f"wg0_{kt}")
                        rows = slice(kt * P, (kt + 1) * P)
                        # x pieces ride sync, w pieces ride scalar: each
                        # (x-f, w-f) pair transfers in parallel instead of
                        # serializing in one ring's FIFO, so the first
                        # matmul's pair completes ~1us earlier.
                        if kt == 0:
                            for f in range(F):
                                nc.sync.dma_start(
                                    xt0[:, f * n:(f + 1) * n],
                                    x_dram[rows, f * n:(f + 1) * n])
                                nc.scalar.dma_start(
                                    wt0[:, f * 2 * HQ:(f + 1) * 2 * HQ],
                                    w13_dram[0, rows,
                                             f * 2 * HQ:(f + 1) * 2 * HQ])
                        else:
                            nc.sync.dma_start(xt0[:, :xh], x_dram[rows, :xh])
                            nc.scalar.dma_start(wt0[:, :wh],
                                                w13_dram[0, rows, :wh])
                            nc.sync.dma_start(xt0[:, xh:], x_dram[rows, xh:])
                            nc.scalar.dma_start(wt0[:, wh:],
                                                w13_dram[0, rows, wh:])
                        x_sb[kt] = xt0
                        wq_sb[0][kt] = wt0
                else:
                    # shared-phase x rides the scalar ring (idle mid-kernel);
                    # weights continue on sync in deadline order
                    for kt in range(KF):
                        t = xpool.tile([P, F * n], bf, tag="x", name=f"xs_{kt}")
                        nc.scalar.dma_start(t[:], x_dram[kt * P:(kt + 1) * P, :])
                        x_sb[kt] = t
                    for kt in range(KF):
                        w = wgpool.tile([P, FW], bf, tag="wg", name=f"sq0_{kt}")
                        nc.sync.dma_start(w[:], w13_dram[0, kt * P:(kt + 1) * P, :])
                        wq_sb[0][kt] = w
                for g in range(1, G):
                    for kt in range(KF):
                        w = wgpool.tile([P, FW], bf, tag="wg",
                                        name=f"wg{g}_{kt}")
                        nc.sync.dma_start(w[:], w13_dram[g, kt * P:(kt + 1) * P, :])
                        wq_sb[g][kt] = w
                for k2 in range(KH):
                    t = w2pool.tile([P, D], bf, tag="w2", name=f"w2_{k2}")
                    nc.sync.dma_start(t[:], w2_dram[k2 * P:(k2 + 1) * P, :])
                    w2_sb[k2] = t

                # GEMM1 over 4 quarters x 4 PSUM banks: quarter g+1's matmuls
                # run in banks 4..7 while quarter g's banks 0..3 evict through
                # ACT silu + DVE mul — no PSUM switch bubbles.
                h_sb = [None] * KH
                for g in range(G):
                    pg1 = [
                        pspool.tile([P, n], f32, tag="ps", name=f"pg1_{g}_{j}")
                        for j in range(JG)
                    ]
                    pg3 = [
                        pspool.tile([P, n], f32, tag="ps", name=f"pg3_{g}_{j}")
                        for j in range(JG)
                    ]
                    # Bank-cycling (j-inner) order: back-to-back matmuls into
                    # the SAME bank cost +53ns each (the 128-cycle drain stops
                    # overlapping the next fill), so rotating across the 4
                    # banks per (kt, f) block is the fast order (+4ns/MM only).
                    for kt in range(KF):
                        wt = wq_sb[g][kt]
                        xt_ = x_sb[kt]
                        for f in range(F):
                            xsl = xt_[:, f * n:(f + 1) * n]
                            first = (kt == 0 and f == 0)
                            last = (kt == KF - 1 and f == F - 1)
                            for j in range(JG):
                                nc.tensor.matmul(
                                    pg1[j][:],
                                    wt[:, f * 2 * HQ + j * P:
                                       f * 2 * HQ + (j + 1) * P],
                                    xsl,
                                    start=first, stop=last,
                                )
                                nc.tensor.matmul(
                                    pg3[j][:],
                                    wt[:, f * 2 * HQ + HQ + j * P:
                                       f * 2 * HQ + HQ + (j + 1) * P],
                                    xsl,
                                    start=first, stop=last,
                                )
                    for j in range(JG):
                        s_sb = hpool.tile([P, n], bf, tag="h")
                        nc.scalar.activation(s_sb[:], pg1[j][:], act.Silu)
                        h = hpool.tile([P, n], bf, tag="h")
                        nc.vector.tensor_mul(h[:], s_sb[:], pg3[j][:])
                        h_sb[g * JG + j] = h

                for gr in range(KF):
                    o = opool.tile([P, F * n], out_dt, tag="o",
                                   name=f"o_{gr}")
                    for fd in range(F):
                        om = fd * KF + gr  # d rows [om*P, om*P+P)
                        po = pspool.tile([P, n], f32, tag="ps",
                                         name=f"po_{om}")
                        for kt in range(KH):
                            nc.tensor.matmul(
                                po[:],
                                w2_sb[kt][:, om * P:(om + 1) * P],
                                h_sb[kt][:],
                                start=(kt == 0), stop=(kt == KH - 1),
                            )
                        nc.vector.tensor_copy(
                            o[:, fd * n:(fd + 1) * n], po[:]
                        )
                        if split_out:
                            # last phase: stream each fold block out as soon
                            # as it is evicted — tail latency beats line
                            # efficiency at kernel end
                            nc.sync.dma_start(
                                out_dram[gr * P:(gr + 1) * P,
                                         fd * n:(fd + 1) * n],
                                o[:, fd * n:(fd + 1) * n],
                            )
                    if not split_out:
                        nc.sync.dma_start(
                            out_dram[gr * P:(gr + 1) * P, :], o[:]
                        )

            warmup()
            ffn(xr, C, w13, w2, o_r, bf, first_phase=True)
            ffn(xs, S, sw13, sw2, o_s, bf, split_out=True)

    nc.compile()
    return nc


def _get_compiled():
    if "nc" not in _COMPILED:
        _COMPILED["nc"] = _build_nc()
        _COMPILED["tmpdir"] = tempfile.mkdtemp(prefix="moe_bass_")
    return _COMPILED["nc"], _COMPILED["tmpdir"]


def _route_host(x, gate, expert_bias):
    """Reference-exact routing on CPU jax: scores, top-2 selection, stable
    sort by expert. Returns (token_idx, expert_ids, scores_sorted) in
    sorted-slot order."""
    import jax
    import jax.numpy as jnp

    cpu = jax.devices("cpu")[0]
    with jax.default_device(cpu):
        xt = jnp.asarray(x.reshape(-1, D))
        scores = jax.nn.sigmoid((xt @ jnp.asarray(gate).T).astype(jnp.float32))
        _, sel = jax.lax.top_k(scores + jnp.asarray(expert_bias)[None, :], TOPK)
        top_scores = jnp.take_along_axis(scores, sel, axis=1) * 1.0
        flat_sel = sel.reshape(-1)
        order = jnp.argsort(flat_sel, stable=True)
        scores_sorted = top_scores.reshape(-1)[order]
        expert_ids = flat_sel[order]
    order = np.asarray(order)
    return (
        order // TOPK,
        np.asarray(expert_ids),
        np.asarray(scores_sorted, dtype=np.float32),
        order,
    )


def _silu32(v):
    return v / (1.0 + np.exp(-v))


def fold_x(x_t):
    # x_t: [D, n] f32/bf16 -> [DR, F*n] bf16, fold-major column blocks
    n = x_t.shape[1]
    xf = np.asarray(x_t).reshape(F, DR, n)
    return np.ascontiguousarray(
        xf.transpose(1, 0, 2).reshape(DR, F * n).astype(BF16)
    )


def unfold_x(arr_f, n_cols):
    # inverse of fold_x: [DR, F*n_cols] -> [D, n_cols]
    out = np.empty((D, n_cols), dtype=arr_f.dtype)
    for f in range(F):
        out[f * DR:(f + 1) * DR] = arr_f[:, f * n_cols:(f + 1) * n_cols]
    return out


def fold_w13(a1, a3):
    # -> [G, DR, F*2*HQ]: per hidden-quarter g, fold-major column blocks,
    # each block = [w1 quarter | w3 quarter]
    out = np.empty((G, DR, F * 2 * HQ), dtype=BF16)
    for g in range(G):
        wg = np.concatenate(
            [a1.T[:, g * HQ:(g + 1) * HQ], a3.T[:, g * HQ:(g + 1) * HQ]],
            axis=1,
        )  # [D, 2*HQ]
        out[g] = wg.reshape(F, DR, 2 * HQ).transpose(1, 0, 2).reshape(
            DR, F * 2 * HQ
        )
    return out


def _overflow_slots_numpy(xb_rows, w1e, w2e, w3e):
    """Exact-math fallback for expert token counts beyond capacity C:
    reproduce the reference's bf16 FFN math in numpy for those rows."""
    a = xb_rows.astype(np.float32)
    g1 = (a @ w1e.astype(BF16).astype(np.float32).T).astype(BF16)
    g3 = (a @ w3e.astype(BF16).astype(np.float32).T).astype(BF16)
    h = (_silu32(g1.astype(np.float32))).astype(BF16).astype(np.float32)
    h = (h * g3.astype(np.float32)).astype(BF16)
    return (h.astype(np.float32) @ w2e.astype(BF16).astype(np.float32).T).astype(
        BF16
    ).astype(np.float32)


def kernel(x, gate, expert_bias, w1, w2, w3, shared_w1, shared_w2, shared_w3):
    global LAST_RESULTS
    from concourse.bass_utils import run_bass_kernel_spmd

    x = np.asarray(x, dtype=np.float32)
    gate = np.asarray(gate, dtype=np.float32)
    expert_bias = np.asarray(expert_bias, dtype=np.float32)
    w1 = np.asarray(w1, dtype=np.float32)
    w2 = np.asarray(w2, dtype=np.float32)
    w3 = np.asarray(w3, dtype=np.float32)
    shared_w1 = np.asarray(shared_w1, dtype=np.float32)
    shared_w2 = np.asarray(shared_w2, dtype=np.float32)
    shared_w3 = np.asarray(shared_w3, dtype=np.float32)

    token_idx, expert_ids, scores_sorted, order = _route_host(x, gate, expert_bias)
    xt = x.reshape(T, D)

    counts = np.bincount(expert_ids, minlength=E)
    offs = np.concatenate([[0], np.cumsum(counts)])

    # Routed tokens, scaled by their gate score then rounded to bf16 exactly
    # like the reference's `routed.astype(bfloat16)`.
    routed_b = (xt[token_idx] * scores_sorted[:, None]).astype(BF16)

    # Shared weights are identical on every core.
    sw13_t = fold_w13(shared_w1, shared_w3)
    sw2_t = np.ascontiguousarray(shared_w2.T.astype(BF16))
    xt_b = xt.astype(BF16)

    in_maps = []
    for e in range(E):
        lo, hi = offs[e], offs[e + 1]
        n_e = min(hi - lo, C)
        xr_t = np.zeros((D, C), dtype=BF16)
        xr_t[:, :n_e] = routed_b[lo:lo + n_e].T
        in_maps.append(
            {
                "xr": fold_x(xr_t),
                "xs": fold_x(xt_b[e * S:(e + 1) * S].T),
                "w13": fold_w13(w1[e], w3[e]),
                "w2": np.ascontiguousarray(w2[e].T.astype(BF16)),
                "sw13": sw13_t,
                "sw2": sw2_t,
            }
        )

    nc, _ = _get_compiled()
    # fresh tmpdir per call: NTFF profile artifacts collide on reuse
    tmpdir = tempfile.mkdtemp(prefix="moe_bass_")
    res = run_bass_kernel_spmd(nc, in_maps, core_ids=list(range(E)), tmpdir=tmpdir)
    LAST_RESULTS = res

    # Reassemble: shared output slices + scatter-add of routed outputs.
    out = np.empty((T, D), dtype=np.float32)
    for e in range(E):
        out[e * S:(e + 1) * S] = unfold_x(res.results[e]["o_s"], S).T

    out_r = np.empty((T * TOPK, D), dtype=np.float32)
    for e in range(E):
        lo, hi = offs[e], offs[e + 1]
        n_e = min(hi - lo, C)
        o_r_e = unfold_x(res.results[e]["o_r"], C)
        out_r[lo:lo + n_e] = o_r_e[:, :n_e].T.astype(np.float32)
        if hi - lo > C:  # capacity overflow: exact numpy fallback
            rows = routed_b[lo + C:hi]
            out_r[lo + C:hi] = _overflow_slots_numpy(rows, w1[e], w2[e], w3[e])

    # slot s (sorted order) came from original flat slot order[s]; invert so
    # each token's two expert outputs can be summed with one gather.
    pos = np.empty(T * TOPK, dtype=np.int64)
    pos[order] = np.arange(T * TOPK)
    out += out_r[pos].reshape(T, TOPK, D).sum(axis=1)

    return out.reshape(4, 512, D)
